# revision 36
# baseline (speedup 1.0000x reference)
"""Action-separated MLP (MoE routing) Trainium2 kernel.

Reference computes all 16 per-action MLPs for every row, then gathers the
selected action's output.  Only the selected expert's output survives, so we
route instead: sort rows by action on the host, run each expert's
512->512->512->1 MLP only on its own rows, and scatter back.  16x fewer FLOPs
than the dense reference.

Distribution: 16 experts over 8 cores, 2 experts per core.  Per-expert row
groups are padded to a common capacity (multiple of the tile width CT) so the
same NEFF runs SPMD on all 8 cores.

Layout: activations are kept transposed (features on SBUF partitions, rows on
the free dim), which makes every layer a plain lhsT.T @ rhs chain with the
per-partition bias + ReLU fused into one ScalarEngine activation op.
Matmuls run as float32r (TF32-like single-pass, 1 cycle/row at N>=256).

Perf notes (from perfetto traces): the kernel is TensorE-bound.  Dummy
warm-up matmuls run during the initial DMA phase so the PE HAM clock-gate is
already at 2.4 GHz when real matmuls start; DMA triggers are ordered so the
first matmul's dependencies land first; CT is picked at runtime to minimize
zero-padding compute; the tiny b3 bias is applied on the host.
"""

import json
import sys

import numpy as np

sys.path.insert(0, "/opt/trn_rl_repo")

import ml_dtypes  # noqa: E402

import concourse.bass as bass  # noqa: E402
import concourse.mybir as mybir  # noqa: E402
import concourse.tile as tile  # noqa: E402

A, D, H = 16, 512, 512
NCORES = 8
EPC = 2  # experts per core
P = 128
KD = D // P  # 4 contraction chunks for layer 1
KH = H // P  # 4 contraction chunks for layers 2/3
N_WARM = 32  # dummy matmuls to warm the PE clock gate during the DMA phase
BPAD = 16    # bias columns at the head of each xt d-chunk (64B-aligned data)
W23C = 528   # w23 tile row stride in columns (64B-aligned chunk bases)

# "bf16": cast inputs to bf16 on host, 1 cycle/row matmuls, least DMA.
# "f32r": fp32 storage, float32r matmuls (1 cycle/row at N>=256).
# "f32":  full fp32 matmuls (4 cycles/row).
DT_MODE = "f32r"


def _split_multiwait_bir(ant_bir_str):
    """This walrus build rejects >1 embedded sync-wait per instruction.
    Move extra waits onto standalone EventSemaphore ops just before the
    owning instruction (same engine, so program order is preserved)."""
    bir = json.loads(
        ant_bir_str.decode() if isinstance(ant_bir_str, bytes) else ant_bir_str
    )
    for fn in bir.get("functions", []):
        for bb in fn.get("blocks", []):
            new_insts = []
            for inst in bb.get("instructions", []):
                si = inst.get("sync_info") or {}
                waits = si.get("on_wait") or []
                if len(waits) > 1:
                    for j, w in enumerate(waits[:-1]):
                        new_insts.append(
                            {
                                "debug": inst.get("debug", 0),
                                "engine": inst["engine"],
                                "ins": [],
                                "name": f"{inst['name']}_xw{j}",
                                "opcode": "EventSemaphore",
                                "outs": [],
                                "sync_info": {"on_update": [], "on_wait": [w]},
                            }
                        )
                    si["on_wait"] = [waits[-1]]
                new_insts.append(inst)
            bb["instructions"] = new_insts
    return json.dumps(bir).encode()


def _install_bir_patch():
    from concourse import bass2jax, bass_utils

    orig = bass_utils.compile_bir_kernel
    if getattr(bass2jax.compile_bir_kernel, "_multiwait_patched", False):
        return

    def patched(ant_bir_str, tmpdir, neff_name="file.neff", **kw):
        return orig(_split_multiwait_bir(ant_bir_str), tmpdir, neff_name=neff_name, **kw)

    patched._multiwait_patched = True
    bass2jax.compile_bir_kernel = patched


def _dtypes():
    if DT_MODE == "bf16":
        return mybir.dt.bfloat16, ml_dtypes.bfloat16
    if DT_MODE == "f32r":
        return mybir.dt.float32r, np.float32
    return mybir.dt.float32, np.float32


def build_nc(c_cap, cts):
    """Build the per-core Bass program: capacity c_cap = sum(cts) per expert."""
    io_dt, _ = _dtypes()
    f32 = mybir.dt.float32
    nt = len(cts)
    offs = [BPAD + sum(cts[:t]) for t in range(nt)]

    nc = bass.Bass()
    # b1/b2 biases ride in the first BPAD columns of each expert's d=0 xt
    # chunk (dedicated [128 x small] DMAs are descriptor-overhead-bound);
    # W3 rides as a 513th column of W2 for the same reason.
    xt_d = nc.dram_tensor("xt", [EPC, KD, P, BPAD + c_cap], io_dt, kind="ExternalInput")
    w1_d = nc.dram_tensor("w1", [EPC, KD, P, H], io_dt, kind="ExternalInput")
    w23_d = nc.dram_tensor("w23", [EPC, KH, P, H + 1], io_dt, kind="ExternalInput")
    y_d = nc.dram_tensor("y", [EPC, c_cap], f32, kind="ExternalOutput")

    RELU = mybir.ActivationFunctionType.Relu

    with tile.TileContext(nc) as tc:
        with (
            tc.tile_pool(name="const", bufs=1) as const,
            tc.tile_pool(name="xt", bufs=2) as xtp,
            tc.tile_pool(name="wts", bufs=2) as wtp,
            tc.tile_pool(name="act", bufs=3) as actp,
            tc.tile_pool(name="out", bufs=4) as outp,
            tc.tile_pool(name="ps", bufs=5, space="PSUM") as psp,
            tc.tile_pool(name="ps3", bufs=2, space="PSUM") as ps3p,
            tc.tile_pool(name="pswarm", bufs=1, space="PSUM") as pswarmp,
        ):
            # PE warm-up: wide dummy matmuls run while the input DMAs
            # stream, so the HAM clock gate (which watches PE *array*
            # activity) un-throttles before the real matmuls start.
            warm_sb = const.tile([P, 64], io_dt, tag="warm")
            nc.vector.memset(warm_sb.bitcast(f32), 0.0)
            warm_ps = pswarmp.tile([64, 64], f32, tag="warm_ps")
            for _ in range(N_WARM):
                nc.tensor.matmul(warm_ps[:], warm_sb[:], warm_sb[:],
                                 start=True, stop=True)

            # DMA issue order and spread matter: one dma_start = one DMA
            # queue (~35 GB/s each), and each trigger costs ~0.7us on its
            # issuing engine.  Split the first tile's inputs into many small
            # transfers across four trigger engines so the PE can start as
            # early as possible; later data streams behind the compute.
            engines = [nc.sync, nc.gpsimd, nc.sync, nc.gpsimd]

            xt_sb = {}
            w_sb = {}
            for e in range(EPC):
                xt_sb[e] = xtp.tile(
                    [P, KD, BPAD + c_cap], io_dt, tag="xt", name=f"xt_sb{e}"
                )
                w1_sb = wtp.tile([P, KD, H], io_dt, tag="w1")
                w23_sb = wtp.tile([P, KH, W23C], io_dt, tag="w23")
                w_sb[e] = (w1_sb, w23_sb)
                # first compute tile's inputs, then this expert's weights
                # (layer-2 weights are needed shortly after layer 1 starts),
                # then the remaining xt tiles
                if e == 0:
                    # Interleave the first matmul group's pieces: xt tile-0
                    # per d-chunk plus only the h=0 column block of w1, so
                    # the group's DMA bytes are minimal.  Later w1 h-blocks,
                    # layer-2/3 weights, and remaining xt tiles follow.
                    for d in range(KD):
                        engines[d].dma_start(
                            xt_sb[e][:, d, : BPAD + cts[0]],
                            xt_d[e, d, :, : BPAD + cts[0]],
                        )
                        engines[(d + 1) % 2].dma_start(
                            w1_sb[:, d, :P], w1_d[e, d, :, :P]
                        )
                    for h in range(1, KH):
                        for d in range(KD):
                            engines[(d + h) % 2].dma_start(
                                w1_sb[:, d, bass.ts(h, P)],
                                w1_d[e, d, :, bass.ts(h, P)],
                            )
                    for d in range(KD):
                        engines[d].dma_start(w23_sb[:, d, : H + 1], w23_d[e, d])
                    for t in range(1, nt):
                        csl = bass.ds(offs[t], cts[t])
                        for d in range(KD):
                            engines[d].dma_start(
                                xt_sb[e][:, d, csl], xt_d[e, d, :, csl]
                            )
                else:
                    for d in range(KD):
                        engines[d].dma_start(
                            xt_sb[e][:, d, : BPAD + cts[0]],
                            xt_d[e, d, :, : BPAD + cts[0]],
                        )
                        engines[(d + 1) % 2].dma_start(w1_sb[:, d, :], w1_d[e, d])
                    for d in range(KD):
                        engines[d].dma_start(w23_sb[:, d, : H + 1], w23_d[e, d])
                    for t in range(1, nt):
                        csl = bass.ds(offs[t], cts[t])
                        for d in range(KD):
                            engines[d].dma_start(
                                xt_sb[e][:, d, csl], xt_d[e, d, :, csl]
                            )

            for e in range(EPC):
                w1_sb, w23_sb = w_sb[e]
                bias = xt_sb[e]
                if io_dt != mybir.dt.bfloat16:
                    bias = bias.bitcast(f32)
                for t in range(nt):
                    ct = cts[t]
                    cs = bass.ds(offs[t], ct)
                    h1_sb = actp.tile([P, KH, max(cts)], io_dt, tag="h1", name="h1_sb")[:, :, :ct]
                    for h in range(KH):
                        ps = psp.tile([P, max(cts)], f32, tag="ps", name="ps")[:, :ct]
                        for d in range(KD):
                            nc.tensor.matmul(
                                ps[:],
                                w1_sb[:, d, bass.ts(h, P)],
                                xt_sb[e][:, d, cs],
                                start=(d == 0),
                                stop=(d == KD - 1),
                            )
                        nc.scalar.activation(
                            h1_sb[:, h, :], ps[:], RELU,
                            bias=bias[:, 0, h : h + 1],
                        )
                    h2_sb = actp.tile([P, KH, max(cts)], io_dt, tag="h2", name="h2_sb")[:, :, :ct]
                    for h in range(KH):
                        ps = psp.tile([P, max(cts)], f32, tag="ps", name="ps")[:, :ct]
                        for d in range(KH):
                            nc.tensor.matmul(
                                ps[:],
                                w23_sb[:, d, bass.ts(h, P)],
                                h1_sb[:, d, :],
                                start=(d == 0),
                                stop=(d == KH - 1),
                            )
                        nc.scalar.activation(
                            h2_sb[:, h, :], ps[:], RELU,
                            bias=bias[:, 0, KH + h : KH + h + 1],
                        )
                    ps3 = ps3p.tile([1, max(cts)], f32, tag="ps3", name="ps3")[:, :ct]
                    for d in range(KH):
                        nc.tensor.matmul(
                            ps3[:],
                            w23_sb[:, d, H : H + 1],
                            h2_sb[:, d, :],
                            start=(d == 0),
                            stop=(d == KH - 1),
                        )
                    y_sb = outp.tile([1, max(cts)], f32, tag="y", name="y_sb")
                    nc.vector.tensor_copy(y_sb[:, :ct], ps3[:])
                    nc.sync.dma_start(
                        y_d[e : e + 1, bass.ds(offs[t] - BPAD, ct)], y_sb[:, :ct]
                    )
    return nc


def _route(state, actions):
    """Sort rows by action; pick per-tile widths at runtime.

    The PE period is LDWEIGHTS-bound, so a narrow first tile costs ~no extra
    PE time but needs far fewer DMA bytes before compute can start."""
    order = np.argsort(actions, kind="stable")
    counts = np.bincount(actions, minlength=A)
    mx = max(int(counts.max()), 1)
    nt = max(1, -(-mx // 512))           # tiles of at most 512 rows
    if nt == 1:
        cts = [max(256, ((mx + 15) // 16) * 16)]
    else:
        rest = -(-(mx - 256) // (nt - 1))
        rest = max(256, ((rest + 15) // 16) * 16)
        if rest > 512:
            nt += 1
            rest = max(256, ((-(-(mx - 256) // (nt - 1)) + 15) // 16) * 16)
        cts = [256] + [rest] * (nt - 1)
    return order, counts, int(sum(cts)), cts


def _build_inputs(state, W1, W2, W3, b1, b2, order, counts, c_cap):
    _, np_dt = _dtypes()
    starts = np.zeros(A + 1, dtype=np.int64)
    starts[1:] = np.cumsum(counts)

    in_maps = []
    for core in range(NCORES):
        es = slice(core * EPC, (core + 1) * EPC)
        xt = np.zeros((EPC, KD, P, BPAD + c_cap), dtype=np_dt)
        for e in range(EPC):
            a = core * EPC + e
            idx = order[starts[a] : starts[a + 1]]
            if len(idx):
                xt[e, :, :, BPAD : BPAD + len(idx)] = (
                    state[idx].T.astype(np_dt).reshape(KD, P, len(idx))
                )
            # bias block: col l*KH+h of the d=0 chunk = b_l[a][h*128:(h+1)*128]
            xt[e, 0, :, 0:KH] = b1[a].astype(np_dt).reshape(KH, P).T
            xt[e, 0, :, KH : 2 * KH] = b2[a].astype(np_dt).reshape(KH, P).T
        w23 = np.concatenate(
            [
                W2[es].reshape(EPC, KH, P, H),
                W3[es].reshape(EPC, KH, P, 1),
            ],
            axis=3,
        )
        in_maps.append(
            {
                "xt": xt,
                "w1": np.ascontiguousarray(W1[es]).astype(np_dt).reshape(EPC, KD, P, H),
                "w23": np.ascontiguousarray(w23).astype(np_dt),
            }
        )
    return in_maps


def _scatter(results, order, counts, b3, actions, B):
    starts = np.zeros(A + 1, dtype=np.int64)
    starts[1:] = np.cumsum(counts)
    out = np.empty((B, 1), dtype=np.float32)
    for core in range(NCORES):
        y = results[core]["y"]
        for e in range(EPC):
            a = core * EPC + e
            idx = order[starts[a] : starts[a + 1]]
            out[idx, 0] = y[e, : len(idx)] + b3[a, 0]
    return out


def run_spmd(nc, in_maps, **kw):
    from concourse.bass_utils import run_bass_kernel_spmd

    _install_bir_patch()
    return run_bass_kernel_spmd(nc, in_maps, core_ids=list(range(NCORES)), **kw)


def prepare(state, W1, b1, W2, b2, W3, b3, actions):
    state = np.asarray(state, dtype=np.float32)
    W1 = np.asarray(W1, dtype=np.float32)
    b1 = np.asarray(b1, dtype=np.float32)
    W2 = np.asarray(W2, dtype=np.float32)
    b2 = np.asarray(b2, dtype=np.float32)
    W3 = np.asarray(W3, dtype=np.float32)
    b3 = np.asarray(b3, dtype=np.float32)
    actions = np.asarray(actions).astype(np.int64)
    order, counts, c_cap, cts = _route(state, actions)
    nc = build_nc(c_cap, cts)
    in_maps = _build_inputs(state, W1, W2, W3, b1, b2, order, counts, c_cap)
    return nc, in_maps, order, counts, b3, actions, state.shape[0]


def kernel(state, W1, b1, W2, b2, W3, b3, actions):
    nc, in_maps, order, counts, b3n, acts, B = prepare(
        state, W1, b1, W2, b2, W3, b3, actions
    )
    res = run_spmd(nc, in_maps)
    return _scatter(res.results, order, counts, b3n, acts, B)


if __name__ == "__main__":
    rng = np.random.default_rng(0)
    B = 4096
    inputs = {
        "state": rng.standard_normal((B, D), dtype=np.float32),
        "W1": rng.standard_normal((A, D, H), dtype=np.float32) / np.sqrt(D),
        "b1": rng.standard_normal((A, H), dtype=np.float32) / np.sqrt(D),
        "W2": rng.standard_normal((A, H, H), dtype=np.float32) / np.sqrt(H),
        "b2": rng.standard_normal((A, H), dtype=np.float32) / np.sqrt(H),
        "W3": rng.standard_normal((A, H, 1), dtype=np.float32) / np.sqrt(H),
        "b3": rng.standard_normal((A, 1), dtype=np.float32) / np.sqrt(H),
        "actions": rng.integers(0, A, B),
    }
    out = kernel(**inputs)
    h1 = np.maximum(
        np.einsum("bd,adh->bah", inputs["state"], inputs["W1"]) + inputs["b1"], 0
    )
    h2 = np.maximum(np.einsum("bah,ahk->bak", h1, inputs["W2"]) + inputs["b2"], 0)
    ref = np.einsum("bah,ahk->bak", h2, inputs["W3"]) + inputs["b3"]
    ref = np.take_along_axis(ref, inputs["actions"][:, None, None], axis=1)[:, 0, :]
    err = np.abs(out - ref).max() / np.abs(ref).max()
    print("self-check rel err:", err)


# revision 37
# speedup vs baseline: 1.0659x; 1.0659x over previous
"""Action-separated MLP (MoE routing) Trainium2 kernel.

Reference computes all 16 per-action MLPs for every row, then gathers the
selected action's output.  Only the selected expert's output survives, so we
route instead: sort rows by action on the host, run each expert's
512->512->512->1 MLP only on its own rows, and scatter back.  16x fewer FLOPs
than the dense reference.

Distribution: 16 experts over 8 cores, 2 experts per core.  Per-expert row
groups are padded to a common capacity (multiple of the tile width CT) so the
same NEFF runs SPMD on all 8 cores.

Layout: activations are kept transposed (features on SBUF partitions, rows on
the free dim), which makes every layer a plain lhsT.T @ rhs chain with the
per-partition bias + ReLU fused into one ScalarEngine activation op.
Matmuls run as float32r (TF32-like single-pass, 1 cycle/row at N>=256).

Perf notes (from perfetto traces): the kernel is TensorE-bound.  Dummy
warm-up matmuls run during the initial DMA phase so the PE HAM clock-gate is
already at 2.4 GHz when real matmuls start; DMA triggers are ordered so the
first matmul's dependencies land first; CT is picked at runtime to minimize
zero-padding compute; the tiny b3 bias is applied on the host.
"""

import json
import sys

import numpy as np

sys.path.insert(0, "/opt/trn_rl_repo")

import ml_dtypes  # noqa: E402

import concourse.bass as bass  # noqa: E402
import concourse.mybir as mybir  # noqa: E402
import concourse.tile as tile  # noqa: E402

A, D, H = 16, 512, 512
NCORES = 8
EPC = 2  # experts per core
P = 128
KD = D // P  # 4 contraction chunks for layer 1
KH = H // P  # 4 contraction chunks for layers 2/3
N_WARM = 32  # dummy matmuls to warm the PE clock gate during the DMA phase
BPAD = 16    # bias columns at the head of each xt d-chunk (64B-aligned data)
W23C = 528   # w23 tile row stride in columns (64B-aligned chunk bases)

# "bf16": cast inputs to bf16 on host, 1 cycle/row matmuls, least DMA.
# "f32r": fp32 storage, float32r matmuls (1 cycle/row at N>=256).
# "f32":  full fp32 matmuls (4 cycles/row).
DT_MODE = "f32r"


def _split_multiwait_bir(ant_bir_str):
    """This walrus build rejects >1 embedded sync-wait per instruction.
    Move extra waits onto standalone EventSemaphore ops just before the
    owning instruction (same engine, so program order is preserved)."""
    bir = json.loads(
        ant_bir_str.decode() if isinstance(ant_bir_str, bytes) else ant_bir_str
    )
    for fn in bir.get("functions", []):
        for bb in fn.get("blocks", []):
            new_insts = []
            for inst in bb.get("instructions", []):
                si = inst.get("sync_info") or {}
                waits = si.get("on_wait") or []
                if len(waits) > 1:
                    for j, w in enumerate(waits[:-1]):
                        new_insts.append(
                            {
                                "debug": inst.get("debug", 0),
                                "engine": inst["engine"],
                                "ins": [],
                                "name": f"{inst['name']}_xw{j}",
                                "opcode": "EventSemaphore",
                                "outs": [],
                                "sync_info": {"on_update": [], "on_wait": [w]},
                            }
                        )
                    si["on_wait"] = [waits[-1]]
                new_insts.append(inst)
            bb["instructions"] = new_insts
    return json.dumps(bir).encode()


def _install_bir_patch():
    from concourse import bass2jax, bass_utils

    orig = bass_utils.compile_bir_kernel
    if getattr(bass2jax.compile_bir_kernel, "_multiwait_patched", False):
        return

    def patched(ant_bir_str, tmpdir, neff_name="file.neff", **kw):
        return orig(_split_multiwait_bir(ant_bir_str), tmpdir, neff_name=neff_name, **kw)

    patched._multiwait_patched = True
    bass2jax.compile_bir_kernel = patched


def _dtypes():
    if DT_MODE == "bf16":
        return mybir.dt.bfloat16, ml_dtypes.bfloat16
    if DT_MODE == "f32r":
        return mybir.dt.float32r, np.float32
    return mybir.dt.float32, np.float32


def build_nc(c_cap, ct):
    """Build the per-core Bass program: capacity c_cap = nt * ct per expert."""
    io_dt, _ = _dtypes()
    f32 = mybir.dt.float32
    nt = c_cap // ct

    nc = bass.Bass()
    # b1/b2 biases ride in the first BPAD columns of each expert's d=0 xt
    # chunk (dedicated [128 x small] DMAs are descriptor-overhead-bound);
    # W3 rides as a 513th column of W2 for the same reason.
    xt_d = nc.dram_tensor("xt", [EPC, KD, P, BPAD + c_cap], io_dt, kind="ExternalInput")
    w1_d = nc.dram_tensor("w1", [EPC, KD, P, H], io_dt, kind="ExternalInput")
    w23_d = nc.dram_tensor("w23", [EPC, KH, P, H + 1], io_dt, kind="ExternalInput")
    y_d = nc.dram_tensor("y", [EPC, c_cap], f32, kind="ExternalOutput")

    RELU = mybir.ActivationFunctionType.Relu

    with tile.TileContext(nc) as tc:
        with (
            tc.tile_pool(name="const", bufs=1) as const,
            tc.tile_pool(name="xt", bufs=2) as xtp,
            tc.tile_pool(name="wts", bufs=2) as wtp,
            tc.tile_pool(name="act", bufs=3) as actp,
            tc.tile_pool(name="out", bufs=4) as outp,
            tc.tile_pool(name="ps", bufs=5, space="PSUM") as psp,
            tc.tile_pool(name="ps3", bufs=2, space="PSUM") as ps3p,
            tc.tile_pool(name="pswarm", bufs=1, space="PSUM") as pswarmp,
        ):
            # PE warm-up: wide dummy matmuls run while the input DMAs
            # stream, so the HAM clock gate (which watches PE *array*
            # activity) un-throttles before the real matmuls start.
            warm_sb = const.tile([P, 64], io_dt, tag="warm")
            nc.vector.memset(warm_sb.bitcast(f32), 0.0)
            warm_ps = pswarmp.tile([64, 64], f32, tag="warm_ps")
            for _ in range(N_WARM):
                nc.tensor.matmul(warm_ps[:], warm_sb[:], warm_sb[:],
                                 start=True, stop=True)

            # DMA issue order and spread matter: one dma_start = one DMA
            # queue (~35 GB/s each), and each trigger costs ~0.7us on its
            # issuing engine.  Split the first tile's inputs into many small
            # transfers across four trigger engines so the PE can start as
            # early as possible; later data streams behind the compute.
            engines = [nc.sync, nc.gpsimd, nc.sync, nc.gpsimd]

            xt_sb = {}
            w_sb = {}
            for e in range(EPC):
                xt_sb[e] = xtp.tile(
                    [P, KD, BPAD + c_cap], io_dt, tag="xt", name=f"xt_sb{e}"
                )
                w1_sb = wtp.tile([P, KD, H], io_dt, tag="w1")
                w23_sb = wtp.tile([P, KH, W23C], io_dt, tag="w23")
                w_sb[e] = (w1_sb, w23_sb)
                # first compute tile's inputs, then this expert's weights
                # (layer-2 weights are needed shortly after layer 1 starts),
                # then the remaining xt tiles
                for d in range(KD):
                    engines[d].dma_start(
                        xt_sb[e][:, d, : BPAD + ct], xt_d[e, d, :, : BPAD + ct]
                    )
                    engines[(d + 1) % 2].dma_start(w1_sb[:, d, :], w1_d[e, d])
                for d in range(KD):
                    engines[d].dma_start(w23_sb[:, d, : H + 1], w23_d[e, d])
                for t in range(1, nt):
                    csl = bass.ds(BPAD + t * ct, ct)
                    for d in range(KD):
                        engines[d].dma_start(xt_sb[e][:, d, csl], xt_d[e, d, :, csl])

            for e in range(EPC):
                w1_sb, w23_sb = w_sb[e]
                bias = xt_sb[e]
                if io_dt != mybir.dt.bfloat16:
                    bias = bias.bitcast(f32)
                for t in range(nt):
                    cs = bass.ds(BPAD + t * ct, ct)
                    h1_sb = actp.tile([P, KH, ct], io_dt, tag="h1")
                    for h in range(KH):
                        ps = psp.tile([P, ct], f32, tag="ps")
                        for d in range(KD):
                            nc.tensor.matmul(
                                ps[:],
                                w1_sb[:, d, bass.ts(h, P)],
                                xt_sb[e][:, d, cs],
                                start=(d == 0),
                                stop=(d == KD - 1),
                            )
                        nc.scalar.activation(
                            h1_sb[:, h, :], ps[:], RELU,
                            bias=bias[:, 0, h : h + 1],
                        )
                    h2_sb = actp.tile([P, KH, ct], io_dt, tag="h2")
                    for h in range(KH):
                        ps = psp.tile([P, ct], f32, tag="ps")
                        for d in range(KH):
                            nc.tensor.matmul(
                                ps[:],
                                w23_sb[:, d, bass.ts(h, P)],
                                h1_sb[:, d, :],
                                start=(d == 0),
                                stop=(d == KH - 1),
                            )
                        nc.scalar.activation(
                            h2_sb[:, h, :], ps[:], RELU,
                            bias=bias[:, 0, KH + h : KH + h + 1],
                        )
                    ps3 = ps3p.tile([1, ct], f32, tag="ps3")
                    for d in range(KH):
                        nc.tensor.matmul(
                            ps3[:],
                            w23_sb[:, d, H : H + 1],
                            h2_sb[:, d, :],
                            start=(d == 0),
                            stop=(d == KH - 1),
                        )
                    y_sb = outp.tile([1, ct], f32, tag="y")
                    nc.vector.tensor_copy(y_sb[:], ps3[:])
                    nc.sync.dma_start(y_d[e : e + 1, bass.ts(t, ct)], y_sb[:])
    return nc


def _route(state, actions):
    """Sort rows by action; pick tile width CT and capacity at runtime."""
    order = np.argsort(actions, kind="stable")
    counts = np.bincount(actions, minlength=A)
    mx = max(int(counts.max()), 1)
    nt = max(1, -(-mx // 512))           # tiles of at most 512 rows
    ct = -(-mx // nt)
    ct = max(256, ((ct + 15) // 16) * 16)  # >=256 keeps float32r at full rate
    return order, counts, nt * ct, ct


def _build_inputs(state, W1, W2, W3, b1, b2, order, counts, c_cap):
    _, np_dt = _dtypes()
    starts = np.zeros(A + 1, dtype=np.int64)
    starts[1:] = np.cumsum(counts)

    in_maps = []
    for core in range(NCORES):
        es = slice(core * EPC, (core + 1) * EPC)
        xt = np.zeros((EPC, KD, P, BPAD + c_cap), dtype=np_dt)
        for e in range(EPC):
            a = core * EPC + e
            idx = order[starts[a] : starts[a + 1]]
            if len(idx):
                xt[e, :, :, BPAD : BPAD + len(idx)] = (
                    state[idx].T.astype(np_dt).reshape(KD, P, len(idx))
                )
            # bias block: col l*KH+h of the d=0 chunk = b_l[a][h*128:(h+1)*128]
            xt[e, 0, :, 0:KH] = b1[a].astype(np_dt).reshape(KH, P).T
            xt[e, 0, :, KH : 2 * KH] = b2[a].astype(np_dt).reshape(KH, P).T
        w23 = np.concatenate(
            [
                W2[es].reshape(EPC, KH, P, H),
                W3[es].reshape(EPC, KH, P, 1),
            ],
            axis=3,
        )
        in_maps.append(
            {
                "xt": xt,
                "w1": np.ascontiguousarray(W1[es]).astype(np_dt).reshape(EPC, KD, P, H),
                "w23": np.ascontiguousarray(w23).astype(np_dt),
            }
        )
    return in_maps


def _scatter(results, order, counts, b3, actions, B):
    starts = np.zeros(A + 1, dtype=np.int64)
    starts[1:] = np.cumsum(counts)
    out = np.empty((B, 1), dtype=np.float32)
    for core in range(NCORES):
        y = results[core]["y"]
        for e in range(EPC):
            a = core * EPC + e
            idx = order[starts[a] : starts[a + 1]]
            out[idx, 0] = y[e, : len(idx)] + b3[a, 0]
    return out


def run_spmd(nc, in_maps, **kw):
    from concourse.bass_utils import run_bass_kernel_spmd

    _install_bir_patch()
    return run_bass_kernel_spmd(nc, in_maps, core_ids=list(range(NCORES)), **kw)


def prepare(state, W1, b1, W2, b2, W3, b3, actions):
    state = np.asarray(state, dtype=np.float32)
    W1 = np.asarray(W1, dtype=np.float32)
    b1 = np.asarray(b1, dtype=np.float32)
    W2 = np.asarray(W2, dtype=np.float32)
    b2 = np.asarray(b2, dtype=np.float32)
    W3 = np.asarray(W3, dtype=np.float32)
    b3 = np.asarray(b3, dtype=np.float32)
    actions = np.asarray(actions).astype(np.int64)
    order, counts, c_cap, ct = _route(state, actions)
    nc = build_nc(c_cap, ct)
    in_maps = _build_inputs(state, W1, W2, W3, b1, b2, order, counts, c_cap)
    return nc, in_maps, order, counts, b3, actions, state.shape[0]


def kernel(state, W1, b1, W2, b2, W3, b3, actions):
    nc, in_maps, order, counts, b3n, acts, B = prepare(
        state, W1, b1, W2, b2, W3, b3, actions
    )
    res = run_spmd(nc, in_maps)
    return _scatter(res.results, order, counts, b3n, acts, B)


if __name__ == "__main__":
    rng = np.random.default_rng(0)
    B = 4096
    inputs = {
        "state": rng.standard_normal((B, D), dtype=np.float32),
        "W1": rng.standard_normal((A, D, H), dtype=np.float32) / np.sqrt(D),
        "b1": rng.standard_normal((A, H), dtype=np.float32) / np.sqrt(D),
        "W2": rng.standard_normal((A, H, H), dtype=np.float32) / np.sqrt(H),
        "b2": rng.standard_normal((A, H), dtype=np.float32) / np.sqrt(H),
        "W3": rng.standard_normal((A, H, 1), dtype=np.float32) / np.sqrt(H),
        "b3": rng.standard_normal((A, 1), dtype=np.float32) / np.sqrt(H),
        "actions": rng.integers(0, A, B),
    }
    out = kernel(**inputs)
    h1 = np.maximum(
        np.einsum("bd,adh->bah", inputs["state"], inputs["W1"]) + inputs["b1"], 0
    )
    h2 = np.maximum(np.einsum("bah,ahk->bak", h1, inputs["W2"]) + inputs["b2"], 0)
    ref = np.einsum("bah,ahk->bak", h2, inputs["W3"]) + inputs["b3"]
    ref = np.take_along_axis(ref, inputs["actions"][:, None, None], axis=1)[:, 0, :]
    err = np.abs(out - ref).max() / np.abs(ref).max()
    print("self-check rel err:", err)


# revision 38
# speedup vs baseline: 1.2484x; 1.1712x over previous
"""Action-separated MLP (MoE routing) Trainium2 kernel.

Reference computes all 16 per-action MLPs for every row, then gathers the
selected action's output.  Only the selected expert's output survives, so we
route instead: sort rows by action on the host, run each expert's
512->512->512->1 MLP only on its own rows, and scatter back.  16x fewer FLOPs
than the dense reference.

Distribution: 16 experts over 8 cores, 2 experts per core.  Per-expert row
groups are padded to a common capacity (multiple of the tile width CT) so the
same NEFF runs SPMD on all 8 cores.

Layout: activations are kept transposed (features on SBUF partitions, rows on
the free dim), which makes every layer a plain lhsT.T @ rhs chain with the
per-partition bias + ReLU fused into one ScalarEngine activation op.
Matmuls run as float32r (TF32-like single-pass, 1 cycle/row at N>=256).

Perf notes (from perfetto traces): the kernel is TensorE-bound.  Dummy
warm-up matmuls run during the initial DMA phase so the PE HAM clock-gate is
already at 2.4 GHz when real matmuls start; DMA triggers are ordered so the
first matmul's dependencies land first; CT is picked at runtime to minimize
zero-padding compute; the tiny b3 bias is applied on the host.
"""

import json
import sys

import numpy as np

sys.path.insert(0, "/opt/trn_rl_repo")

import ml_dtypes  # noqa: E402

import concourse.bass as bass  # noqa: E402
import concourse.mybir as mybir  # noqa: E402
import concourse.tile as tile  # noqa: E402

A, D, H = 16, 512, 512
NCORES = 8
EPC = 2  # experts per core
P = 128
KD = D // P  # 4 contraction chunks for layer 1
KH = H // P  # 4 contraction chunks for layers 2/3
N_WARM = 32  # dummy matmuls to warm the PE clock gate during the DMA phase
BPAD = 16    # bias columns at the head of each xt d-chunk (64B-aligned data)
W23C = 528   # w23 tile row stride in columns (64B-aligned chunk bases)

# "bf16": cast inputs to bf16 on host, 1 cycle/row matmuls, least DMA.
# "f32r": fp32 storage, float32r matmuls (1 cycle/row at N>=256).
# "f32":  full fp32 matmuls (4 cycles/row).
DT_MODE = "f32r"


def _split_multiwait_bir(ant_bir_str):
    """This walrus build rejects >1 embedded sync-wait per instruction.
    Move extra waits onto standalone EventSemaphore ops just before the
    owning instruction (same engine, so program order is preserved)."""
    bir = json.loads(
        ant_bir_str.decode() if isinstance(ant_bir_str, bytes) else ant_bir_str
    )
    for fn in bir.get("functions", []):
        for bb in fn.get("blocks", []):
            new_insts = []
            for inst in bb.get("instructions", []):
                si = inst.get("sync_info") or {}
                waits = si.get("on_wait") or []
                if len(waits) > 1:
                    for j, w in enumerate(waits[:-1]):
                        new_insts.append(
                            {
                                "debug": inst.get("debug", 0),
                                "engine": inst["engine"],
                                "ins": [],
                                "name": f"{inst['name']}_xw{j}",
                                "opcode": "EventSemaphore",
                                "outs": [],
                                "sync_info": {"on_update": [], "on_wait": [w]},
                            }
                        )
                    si["on_wait"] = [waits[-1]]
                new_insts.append(inst)
            bb["instructions"] = new_insts
    return json.dumps(bir).encode()


def _install_bir_patch():
    from concourse import bass2jax, bass_utils

    orig = bass_utils.compile_bir_kernel
    if getattr(bass2jax.compile_bir_kernel, "_multiwait_patched", False):
        return

    def patched(ant_bir_str, tmpdir, neff_name="file.neff", **kw):
        return orig(_split_multiwait_bir(ant_bir_str), tmpdir, neff_name=neff_name, **kw)

    patched._multiwait_patched = True
    bass2jax.compile_bir_kernel = patched


def _dtypes():
    if DT_MODE == "bf16":
        return mybir.dt.bfloat16, ml_dtypes.bfloat16
    if DT_MODE == "f32r":
        return mybir.dt.float32r, np.float32
    return mybir.dt.float32, np.float32


def build_nc(c_cap, cts):
    """Build the per-core Bass program: capacity c_cap = sum(cts) per expert."""
    io_dt, _ = _dtypes()
    f32 = mybir.dt.float32
    nt = len(cts)
    offs = [BPAD + sum(cts[:t]) for t in range(nt)]

    nc = bass.Bass()
    # b1/b2 biases ride in the first BPAD columns of each expert's d=0 xt
    # chunk (dedicated [128 x small] DMAs are descriptor-overhead-bound);
    # W3 rides as a 513th column of W2 for the same reason.
    xt_d = nc.dram_tensor("xt", [EPC, KD, P, BPAD + c_cap], io_dt, kind="ExternalInput")
    w1_d = nc.dram_tensor("w1", [EPC, KD, P, H], io_dt, kind="ExternalInput")
    w23_d = nc.dram_tensor("w23", [EPC, KH, P, H + 1], io_dt, kind="ExternalInput")
    y_d = nc.dram_tensor("y", [EPC, c_cap], f32, kind="ExternalOutput")

    RELU = mybir.ActivationFunctionType.Relu

    with tile.TileContext(nc) as tc:
        with (
            tc.tile_pool(name="const", bufs=1) as const,
            tc.tile_pool(name="xt", bufs=2) as xtp,
            tc.tile_pool(name="wts", bufs=2) as wtp,
            tc.tile_pool(name="act", bufs=3) as actp,
            tc.tile_pool(name="out", bufs=4) as outp,
            tc.tile_pool(name="ps", bufs=5, space="PSUM") as psp,
            tc.tile_pool(name="ps3", bufs=2, space="PSUM") as ps3p,
            tc.tile_pool(name="pswarm", bufs=1, space="PSUM") as pswarmp,
        ):
            # PE warm-up: wide dummy matmuls run while the input DMAs
            # stream, so the HAM clock gate (which watches PE *array*
            # activity) un-throttles before the real matmuls start.
            warm_sb = const.tile([P, 64], io_dt, tag="warm")
            nc.vector.memset(warm_sb.bitcast(f32), 0.0)
            warm_ps = pswarmp.tile([64, 64], f32, tag="warm_ps")
            for _ in range(N_WARM):
                nc.tensor.matmul(warm_ps[:], warm_sb[:], warm_sb[:],
                                 start=True, stop=True)

            # DMA issue order and spread matter: one dma_start = one DMA
            # queue (~35 GB/s each), and each trigger costs ~0.7us on its
            # issuing engine.  Split the first tile's inputs into many small
            # transfers across four trigger engines so the PE can start as
            # early as possible; later data streams behind the compute.
            engines = [nc.sync, nc.gpsimd, nc.sync, nc.gpsimd]

            xt_sb = {}
            w_sb = {}
            for e in range(EPC):
                xt_sb[e] = xtp.tile(
                    [P, KD, BPAD + c_cap], io_dt, tag="xt", name=f"xt_sb{e}"
                )
                w1_sb = wtp.tile([P, KD, H], io_dt, tag="w1")
                w23_sb = wtp.tile([P, KH, W23C], io_dt, tag="w23")
                w_sb[e] = (w1_sb, w23_sb)
                # first compute tile's inputs, then this expert's weights
                # (layer-2 weights are needed shortly after layer 1 starts),
                # then the remaining xt tiles
                if e == 0:
                    # Interleave the first matmul group's pieces: xt tile-0
                    # per d-chunk plus only the h=0 column block of w1, so
                    # the group's DMA bytes are minimal.  Later w1 h-blocks,
                    # layer-2/3 weights, and remaining xt tiles follow.
                    for d in range(KD):
                        engines[d].dma_start(
                            xt_sb[e][:, d, : BPAD + cts[0]],
                            xt_d[e, d, :, : BPAD + cts[0]],
                        )
                        engines[(d + 1) % 2].dma_start(
                            w1_sb[:, d, : H // 2], w1_d[e, d, :, : H // 2]
                        )
                    for d in range(KD):
                        engines[d % 2].dma_start(
                            w1_sb[:, d, H // 2 :], w1_d[e, d, :, H // 2 :]
                        )
                    for d in range(KD):
                        engines[d].dma_start(w23_sb[:, d, : H + 1], w23_d[e, d])
                    for t in range(1, nt):
                        csl = bass.ds(offs[t], cts[t])
                        for d in range(KD):
                            engines[d].dma_start(
                                xt_sb[e][:, d, csl], xt_d[e, d, :, csl]
                            )
                else:
                    for d in range(KD):
                        engines[d].dma_start(
                            xt_sb[e][:, d, : BPAD + cts[0]],
                            xt_d[e, d, :, : BPAD + cts[0]],
                        )
                        engines[(d + 1) % 2].dma_start(w1_sb[:, d, :], w1_d[e, d])
                    for d in range(KD):
                        engines[d].dma_start(w23_sb[:, d, : H + 1], w23_d[e, d])
                    for t in range(1, nt):
                        csl = bass.ds(offs[t], cts[t])
                        for d in range(KD):
                            engines[d].dma_start(
                                xt_sb[e][:, d, csl], xt_d[e, d, :, csl]
                            )

            for e in range(EPC):
                w1_sb, w23_sb = w_sb[e]
                bias = xt_sb[e]
                if io_dt != mybir.dt.bfloat16:
                    bias = bias.bitcast(f32)
                for t in range(nt):
                    ct = cts[t]
                    cs = bass.ds(offs[t], ct)
                    h1_sb = actp.tile([P, KH, max(cts)], io_dt, tag="h1", name="h1_sb")[:, :, :ct]
                    for h in range(KH):
                        ps = psp.tile([P, max(cts)], f32, tag="ps", name="ps")[:, :ct]
                        for d in range(KD):
                            nc.tensor.matmul(
                                ps[:],
                                w1_sb[:, d, bass.ts(h, P)],
                                xt_sb[e][:, d, cs],
                                start=(d == 0),
                                stop=(d == KD - 1),
                            )
                        nc.scalar.activation(
                            h1_sb[:, h, :], ps[:], RELU,
                            bias=bias[:, 0, h : h + 1],
                        )
                    h2_sb = actp.tile([P, KH, max(cts)], io_dt, tag="h2", name="h2_sb")[:, :, :ct]
                    for h in range(KH):
                        ps = psp.tile([P, max(cts)], f32, tag="ps", name="ps")[:, :ct]
                        for d in range(KH):
                            nc.tensor.matmul(
                                ps[:],
                                w23_sb[:, d, bass.ts(h, P)],
                                h1_sb[:, d, :],
                                start=(d == 0),
                                stop=(d == KH - 1),
                            )
                        nc.scalar.activation(
                            h2_sb[:, h, :], ps[:], RELU,
                            bias=bias[:, 0, KH + h : KH + h + 1],
                        )
                    ps3 = ps3p.tile([1, max(cts)], f32, tag="ps3", name="ps3")[:, :ct]
                    for d in range(KH):
                        nc.tensor.matmul(
                            ps3[:],
                            w23_sb[:, d, H : H + 1],
                            h2_sb[:, d, :],
                            start=(d == 0),
                            stop=(d == KH - 1),
                        )
                    y_sb = outp.tile([1, max(cts)], f32, tag="y", name="y_sb")
                    nc.vector.tensor_copy(y_sb[:, :ct], ps3[:])
                    nc.sync.dma_start(
                        y_d[e : e + 1, bass.ds(offs[t] - BPAD, ct)], y_sb[:, :ct]
                    )
    return nc


def _route(state, actions):
    """Sort rows by action; pick per-tile widths at runtime.

    The PE period is LDWEIGHTS-bound, so a narrow first tile costs ~no extra
    PE time but needs far fewer DMA bytes before compute can start."""
    order = np.argsort(actions, kind="stable")
    counts = np.bincount(actions, minlength=A)
    mx = max(int(counts.max()), 1)
    nt = max(1, -(-mx // 512))           # tiles of at most 512 rows
    if nt == 1:
        cts = [max(256, ((mx + 15) // 16) * 16)]
    else:
        rest = -(-(mx - 256) // (nt - 1))
        rest = max(256, ((rest + 15) // 16) * 16)
        if rest > 512:
            nt += 1
            rest = max(256, ((-(-(mx - 256) // (nt - 1)) + 15) // 16) * 16)
        cts = [256] + [rest] * (nt - 1)
    return order, counts, int(sum(cts)), cts


def _build_inputs(state, W1, W2, W3, b1, b2, order, counts, c_cap):
    _, np_dt = _dtypes()
    starts = np.zeros(A + 1, dtype=np.int64)
    starts[1:] = np.cumsum(counts)

    in_maps = []
    for core in range(NCORES):
        es = slice(core * EPC, (core + 1) * EPC)
        xt = np.zeros((EPC, KD, P, BPAD + c_cap), dtype=np_dt)
        for e in range(EPC):
            a = core * EPC + e
            idx = order[starts[a] : starts[a + 1]]
            if len(idx):
                xt[e, :, :, BPAD : BPAD + len(idx)] = (
                    state[idx].T.astype(np_dt).reshape(KD, P, len(idx))
                )
            # bias block: col l*KH+h of the d=0 chunk = b_l[a][h*128:(h+1)*128]
            xt[e, 0, :, 0:KH] = b1[a].astype(np_dt).reshape(KH, P).T
            xt[e, 0, :, KH : 2 * KH] = b2[a].astype(np_dt).reshape(KH, P).T
        w23 = np.concatenate(
            [
                W2[es].reshape(EPC, KH, P, H),
                W3[es].reshape(EPC, KH, P, 1),
            ],
            axis=3,
        )
        in_maps.append(
            {
                "xt": xt,
                "w1": np.ascontiguousarray(W1[es]).astype(np_dt).reshape(EPC, KD, P, H),
                "w23": np.ascontiguousarray(w23).astype(np_dt),
            }
        )
    return in_maps


def _scatter(results, order, counts, b3, actions, B):
    starts = np.zeros(A + 1, dtype=np.int64)
    starts[1:] = np.cumsum(counts)
    out = np.empty((B, 1), dtype=np.float32)
    for core in range(NCORES):
        y = results[core]["y"]
        for e in range(EPC):
            a = core * EPC + e
            idx = order[starts[a] : starts[a + 1]]
            out[idx, 0] = y[e, : len(idx)] + b3[a, 0]
    return out


def run_spmd(nc, in_maps, **kw):
    from concourse.bass_utils import run_bass_kernel_spmd

    _install_bir_patch()
    return run_bass_kernel_spmd(nc, in_maps, core_ids=list(range(NCORES)), **kw)


def prepare(state, W1, b1, W2, b2, W3, b3, actions):
    state = np.asarray(state, dtype=np.float32)
    W1 = np.asarray(W1, dtype=np.float32)
    b1 = np.asarray(b1, dtype=np.float32)
    W2 = np.asarray(W2, dtype=np.float32)
    b2 = np.asarray(b2, dtype=np.float32)
    W3 = np.asarray(W3, dtype=np.float32)
    b3 = np.asarray(b3, dtype=np.float32)
    actions = np.asarray(actions).astype(np.int64)
    order, counts, c_cap, cts = _route(state, actions)
    nc = build_nc(c_cap, cts)
    in_maps = _build_inputs(state, W1, W2, W3, b1, b2, order, counts, c_cap)
    return nc, in_maps, order, counts, b3, actions, state.shape[0]


def kernel(state, W1, b1, W2, b2, W3, b3, actions):
    nc, in_maps, order, counts, b3n, acts, B = prepare(
        state, W1, b1, W2, b2, W3, b3, actions
    )
    res = run_spmd(nc, in_maps)
    return _scatter(res.results, order, counts, b3n, acts, B)


if __name__ == "__main__":
    rng = np.random.default_rng(0)
    B = 4096
    inputs = {
        "state": rng.standard_normal((B, D), dtype=np.float32),
        "W1": rng.standard_normal((A, D, H), dtype=np.float32) / np.sqrt(D),
        "b1": rng.standard_normal((A, H), dtype=np.float32) / np.sqrt(D),
        "W2": rng.standard_normal((A, H, H), dtype=np.float32) / np.sqrt(H),
        "b2": rng.standard_normal((A, H), dtype=np.float32) / np.sqrt(H),
        "W3": rng.standard_normal((A, H, 1), dtype=np.float32) / np.sqrt(H),
        "b3": rng.standard_normal((A, 1), dtype=np.float32) / np.sqrt(H),
        "actions": rng.integers(0, A, B),
    }
    out = kernel(**inputs)
    h1 = np.maximum(
        np.einsum("bd,adh->bah", inputs["state"], inputs["W1"]) + inputs["b1"], 0
    )
    h2 = np.maximum(np.einsum("bah,ahk->bak", h1, inputs["W2"]) + inputs["b2"], 0)
    ref = np.einsum("bah,ahk->bak", h2, inputs["W3"]) + inputs["b3"]
    ref = np.take_along_axis(ref, inputs["actions"][:, None, None], axis=1)[:, 0, :]
    err = np.abs(out - ref).max() / np.abs(ref).max()
    print("self-check rel err:", err)
